# revision 4
# baseline (speedup 1.0000x reference)
"""Central-difference L1 loss kernel for 8 trn2 NeuronCores.

Math: with d = x - y, the loss is
    mean_{27 offsets o} |d[v] - d_pad[v + o]|
over the (B,C,D,H,W) = (2,1,32,128,128) volume, zero-padded by 1 in D/H/W.

Offset symmetry: |d[v] - d[v+o]| is counted by both o and -o, so only 13
canonical offsets (first nonzero of (oh, od, ow) positive) are computed on
device; total = 2 * sum(canonical directed sums) + sum_v gamma(v) * |d[v]|,
where gamma is a small integer boundary weight, constant on the 27 cells of
(d-class, h-class, w-class). The device also emits 9 per-partition |d|
region sums (slice-class x w-class; h = partition gives free h resolution);
the host applies gamma and folds in float64.

Sharding: 8 shards over (B=2) x (D in 4 chunks of 8 slices). Each core gets a
[128(H), 2(x|y), 10(slices incl halo), 132(W incl pad)] fp32 slab with zeros
in halo/pad positions that fall outside the volume.

Device per core (bf16 pipeline):
  d0   = x - y  (bf16; DVE tensor_tensor, one per slab half)
  d0s  = d0 shifted by one flat element (ScalarE copy; keeps the packed-bf16
         pair alignment so the custom DVE op runs its 2x_1p mode for w+-1)
  d0h  = d0 shifted by one partition (SBUF->SBUF DMA; h+1 neighbor)
  d0sh = d0s shifted by one partition (SBUF->SBUF DMA)
  13 canonical passes, one custom DVE op each (ABS_DIFF_ACC, hardware
  accumulator chained across ops), + 1 tiny flush op -> acc[:, 0]
  9 ScalarE activation(Abs) region sums with accum_out -> acc[:, 1..9]
Host folds 8 x [128, 10] partials with gamma weights in float64.
"""

import numpy as np

# ---- problem constants (hardcoded; kernel.py must be self-contained) ----
B, C, D, H, W = 2, 1, 32, 128, 128
N_CORES = 8
D_CHUNK = D // 4  # 8 slices per core
SLAB_S = D_CHUNK + 2  # with halo
SLAB_W = W + 4  # W + 2 pad each side (keeps slice stride & data start even)
FLAT = SLAB_S * SLAB_W
HALF = (SLAB_S // 2) * SLAB_W  # flat size of one slab half (5 slices)
N_OFFSETS = 27
TOTAL_COUNT = N_OFFSETS * B * C * D * H * W

# canonical offsets (od, oh, ow): first nonzero of (oh, od, ow) is +1,
# ordered by buffer readiness: d0 -> d0s -> d0h -> d0sh
PASSES = (
    [(1, 0, 0)]
    + [(1, 0, -1), (1, 0, 1), (0, 0, 1)]
    + [(-1, 1, 0), (0, 1, 0), (1, 1, 0)]
    + [(od, 1, ow) for od in (-1, 0, 1) for ow in (-1, 1)]
)
assert len(PASSES) == 13

_cached = None
_ABS_OP = None


def _register_abs_diff_op():
    """Register two custom DVE op rows:
      ABS2X_SEED: seed (acc <- 0) + steady; ABS2X_CONT: steady only (the
    hardware accumulator keeps integrating across instructions).
    Steady body (both rows, both modes) uses the native v3 ABSOLUTE_DIFF op:
      1x: |a - b| per element; 2x: |a-b| of the packed lo+hi bf16 pair summed.
    Machine shape throughout: accumulate recurrence early (CURR_ALU_OUT), acc
    rides the BYPASS chain with a_flop re-latched on every block to the end;
    DVE_READ_ACCUMULATOR2 taps that chain. The read only decodes correctly
    when the op's dst dtype is fp32, so the hot bf16 passes skip accum_out and
    a final tiny fp32-dst flush op (in0 == in1, adds 0) extracts the total."""
    global _ABS_OP
    if _ABS_OP is not None:
        return _ABS_OP
    from dataclasses import dataclass
    from operator import add

    import concourse.dve_ops as dve_ops
    from concourse.dve_ops import OPS, CUSTOM_DVE_SPECS, DveOp
    from concourse.dve_spec import Spec, Src0, Src1, lower, maxx
    from concourse.dve_uop import (
        AluInp,
        AluOp,
        DelayInp,
        DveOpSpec,
        InpSel,
        OutPath,
        OutSel,
        Trigger,
        UopConfig,
        UopDpConfig,
    )

    def _ref(in0, in1, s0, s1, imm2):
        b = np.abs(in0.astype(np.float32) - in1.astype(np.float32))
        return b, b.reshape(b.shape[0], -1).sum(axis=-1, keepdims=True)

    spec = Spec(body=maxx(Src0 - Src1, Src1 - Src0), accum=add, reference=_ref)

    PA, CA = AluInp.PREV_ALU_OUT, AluInp.CURR_ALU_OUT
    PD = lambda n: AluInp(int(AluInp.PREV_DELAY_0) + n)

    def mk_uop(kind, two_x):
        INP = [
            InpSel.SRC_0,
            InpSel.SRC_1,
            InpSel.SRC_0_HI if two_x else InpSel.ZERO,
            InpSel.SRC_1_HI if two_x else InpSel.ZERO,
        ] + [InpSel.ZERO] * 4
        INP_EN = ([1, 1, 1, 1] if two_x else [1, 1, 0, 0]) + [0, 0, 0, 0]
        bs = []
        for _ in range(8):
            b = UopDpConfig()
            b.op, b.alu_src0, b.alu_src1 = AluOp.BYPASS, PA, PA
            b.alu_out_enable = 1
            bs.append(b)

        def alu(i, op, s0, s1):
            bs[i].op, bs[i].alu_src0, bs[i].alu_src1 = op, s0, s1

        def chain(i, n, src=DelayInp.PREV_DELAY):
            bs[i].delay[n] = src
            bs[i].delay_enable[n] = 1

        if kind == "seed":
            acc_stage = 3
            alu(3, AluOp.BITWISE_XOR, PA, PA)  # acc <- 0
        elif two_x:
            acc_stage = 3
            alu(0, AluOp.ABSOLUTE_DIFF, PA, PD(0))  # |a_lo - b_lo|
            alu(1, AluOp.ABSOLUTE_DIFF, PD(1), PD(2))  # |a_hi - b_hi|
            alu(2, AluOp.ADD, PA, PD(3))  # pair sum
            alu(3, AluOp.ADD, CA, PA)  # accumulate
            chain(0, 1)  # a_hi to blk1
            chain(0, 2)  # b_hi to blk1
            chain(1, 3, DelayInp.PREV_ALU_OUT)  # chain3 <- |d_lo|
            chain(3, 0, DelayInp.PREV_ALU_OUT)  # chain0 <- body (for out)
            for i in (4, 5, 6, 7):
                chain(i, 0)
        else:
            # accum stage MUST match the 2x program (block 3): the running
            # total lives in that block's out-flop across chained ops, and a
            # mode-mismatched op in the chain must find it in the same place
            acc_stage = 3
            alu(0, AluOp.ABSOLUTE_DIFF, PA, PD(0))  # |a - b|
            alu(3, AluOp.ADD, CA, PA)  # accumulate
            chain(3, 0, DelayInp.PREV_ALU_OUT)  # chain0 <- body (for out)
            for i in (4, 5, 6, 7):
                chain(i, 0)
        for i in range(acc_stage, 8):
            bs[i].alu_out_a_enable = 1
        u = UopConfig(
            datapath_config=bs,
            inp=list(INP),
            inp_enable=list(INP_EN),
            accum_enabled=1,
            require_inp0=0 if kind == "seed" else 1,
            require_inp1=0 if kind == "seed" else 1,
            trigger=(
                (Trigger.COUNT, Trigger.NONE, Trigger.NONE)
                if kind == "seed"
                else (Trigger.SRC_TENSOR_DONE, Trigger.NONE, Trigger.NONE)
            ),
            next_uop=(1, 0, 0) if kind == "seed" else (0, 0, 0),
            repeat_count=1 if kind == "seed" else 0,
        )
        if kind != "seed":
            u.out[OutPath.WR0_LO] = OutSel.DELAY_0
            u.out_enable[OutPath.WR0_LO] = 1
            if two_x:
                u.out[OutPath.WR0_HI] = OutSel.DELAY_0
                u.out_enable[OutPath.WR0_HI] = 1
        return u

    def register(name, with_seed):
        row = max(dve_ops._SUB_OPCODE_FOR_NAME.values()) + 1
        assert row < 0x20
        dve_ops._SUB_OPCODE_FOR_NAME[name] = row

        if with_seed:
            u1 = [mk_uop("seed", False), mk_uop("steady", False)]
            u2 = [mk_uop("seed", True), mk_uop("steady", True)]
        else:
            u1 = [mk_uop("steady", False)]
            u2 = [mk_uop("steady", True)]

        @dataclass(frozen=True)
        class DveOpHand(DveOp):
            def compile(self, ver):
                key = (self.name, ver)
                if (r := dve_ops._COMPILE_CACHE.get(key)) is not None:
                    return r
                if ver == "v3":
                    r = DveOpSpec(
                        name=self.name, opcode=row, uops=u1, uops_2x=u2,
                        rd1_en=True, perf_max=1,
                    )
                else:
                    r = DveOpSpec(
                        name=self.name, opcode=row,
                        uops=lower(spec, ver=ver), rd1_en=True,
                    )
                dve_ops._COMPILE_CACHE[key] = r
                return r

        op = DveOpHand(name, spec, subdim=False, uops_sha={})
        OPS.append(op)
        CUSTOM_DVE_SPECS[name] = spec
        return op

    _ABS_OP = (register("ABS2X_SEED_V7_ANT", True), register("ABS2X_CONT_V7_ANT", False))
    return _ABS_OP


def _emit_abs(nc, op, out, in0, in1, accum_out=None, s0=0.0):
    """_custom_dve clone that sets perf_max=1 (byte-36[7:6]) so the engine
    picks the 2x_1p uop slot when the APs qualify (silent 1x fallback)."""
    import concourse.bass_isa as bass_isa
    from concourse import mybir
    from concourse.dve_ops import get_dve_sub_opcode

    v = nc.vector
    if op.name not in nc.m.ant_custom_dve_ops:
        nc.m.ant_custom_dve_ops = sorted({*nc.m.ant_custom_dve_ops, op.name})
    shape = bass_isa.CustomDveShape.STT
    isa_opcode = nc.isa.Opcode[
        f"NEURON_ISA_TPB_OPCODE_CUSTOM_DVE_ANT_{shape.slot()}"
    ].value
    zero = mybir.ImmediateValue(dtype=mybir.dt.float32, value=0.0)
    s0_l = v.lower_ap(s0, for_isa=True) if not isinstance(s0, float) else zero
    ins = [
        v.lower_ap(in0, for_isa=True, opt=True),
        v.lower_ap(in1, for_isa=True, opt=True),
        s0_l,
        zero,
    ]
    outs = [v.lower_ap(out, for_isa=True, opt=True)]
    if accum_out is not None:
        outs.append(v.lower_ap(accum_out, for_isa=True))
    return v.add_instruction(
        bass_isa.InstCustomDveAnt(
            name=nc.get_next_instruction_name(),
            op_name=op.name,
            rd1_en=True,
            subdim=0,
            imm2=0.0,
            shape=shape,
            row=get_dve_sub_opcode(op.name),
            isa_opcode=isa_opcode,
            ins=ins,
            outs=outs,
            perf_max=1,
        )
    )


def _build():
    """Build and schedule the Bass program once; return (nc, out_name)."""
    import concourse.tile as tile
    from concourse import bacc, mybir

    seed_op, cont_op = _register_abs_diff_op()
    f32 = mybir.dt.float32
    bf16 = mybir.dt.bfloat16
    AF = mybir.ActivationFunctionType
    nc = bacc.Bacc(
        "TRN2",
        target_bir_lowering=False,
        debug=False,
        enable_asserts=False,
        num_devices=N_CORES,
    )
    xy = nc.dram_tensor("xy", [H, 2, 2, SLAB_S // 2, SLAB_W], f32, kind="ExternalInput").ap()
    out = nc.dram_tensor("out", [H, 10], f32, kind="ExternalOutput").ap()

    with tile.TileContext(nc) as tc:
        with (
            tc.tile_pool(name="main", bufs=1) as pool,
        ):
            xyt = pool.tile([H, 2, 2, SLAB_S // 2, SLAB_W], f32)
            d0 = pool.tile([H, SLAB_S, SLAB_W], bf16)
            d0s = pool.tile([H, SLAB_S, SLAB_W], bf16)
            d0h = pool.tile([H, SLAB_S, SLAB_W], bf16)
            d0sh = pool.tile([H, SLAB_S, SLAB_W], bf16)
            acc = pool.tile([H, 10], f32)
            dve_sc = pool.tile([H, D_CHUNK, W], bf16)  # shared scrap: WAW chain
            reg_sc = pool.tile([H, SLAB_S - 4, W - 2], bf16)  # scalar scrap

            # input DMAs: 4 chunks spread across engine DGE rings so the
            # transfers run in parallel
            nc.sync.dma_start(xyt[:, 0, 0], xy[:, 0, 0])
            nc.scalar.dma_start(xyt[:, 0, 1], xy[:, 0, 1])
            nc.gpsimd.dma_start(xyt[:, 1, 0], xy[:, 1, 0])
            nc.sync.dma_start(xyt[:, 1, 1], xy[:, 1, 1])

            half = SLAB_S // 2
            for k, (s0_, s1_) in enumerate(((0, half), (half, SLAB_S))):
                nc.vector.tensor_tensor(
                    out=d0[:, s0_:s1_],
                    in0=xyt[:, k, 0],
                    in1=xyt[:, k, 1],
                    op=mybir.AluOpType.subtract,
                )
            d0f = d0[:].rearrange("p a b -> p (a b)")
            d0sf = d0s[:].rearrange("p a b -> p (a b)")
            d0hf = d0h[:].rearrange("p a b -> p (a b)")
            d0shf = d0sh[:].rearrange("p a b -> p (a b)")

            # d0s: flat shift by one element (ScalarE), split per slab half so
            # the first chunk only waits on d0's first half
            nc.scalar.copy(d0sf[:, 0 : HALF - 1], d0f[:, 1:HALF])
            nc.scalar.copy(d0sf[:, HALF - 1 : FLAT - 1], d0f[:, HALF:FLAT])

            # h+1 twins: SBUF->SBUF DMA shifted by one partition, chunked per
            # half, on otherwise-idle engine rings
            nc.sync.dma_start(d0hf[0:127, 0:HALF], d0f[1:128, 0:HALF])
            nc.sync.dma_start(d0hf[0:127, HALF:FLAT], d0f[1:128, HALF:FLAT])
            nc.gpsimd.dma_start(d0shf[0:127, 0:HALF], d0sf[1:128, 0:HALF])
            nc.gpsimd.dma_start(d0shf[0:127, HALF:FLAT], d0sf[1:128, HALF:FLAT])

            # 13 canonical passes on the DVE, one hardware-accumulator chain
            for i, (od, oh, ow) in enumerate(PASSES):
                np_ = 127 if oh == 1 else 128
                if ow == 0:
                    twin = d0h if oh == 1 else d0
                    c0, c1 = 2, 130
                else:
                    twin = d0sh if oh == 1 else d0s
                    c0, c1 = (2, 130) if ow == 1 else (0, 128)
                in0 = d0[0:np_, 1:9, 2:130]
                in1 = twin[0:np_, 1 + od : 9 + od, c0:c1]
                op = seed_op if i == 0 else cont_op
                _emit_abs(nc, op, dve_sc[0:np_], in0, in1)

            # 9 |d| region sums on ScalarE (slice-class x w-class), per-
            # partition accumulators; host applies gamma weights
            ridx = 0
            for s0_, s1_ in ((1, 2), (2, 8), (8, 9)):
                for c0, c1 in ((2, 3), (3, 129), (129, 130)):
                    nc.scalar.activation(
                        reg_sc[:, 0 : s1_ - s0_, 0 : c1 - c0],
                        d0[:, s0_:s1_, c0:c1],
                        AF.Abs,
                        accum_out=acc[:, 1 + ridx : 2 + ridx],
                    )
                    ridx += 1

            # flush: tiny fp32-dst continue op; in0 == in1 adds 0; its
            # appended accumulator read decodes correctly (fp32) and lands
            # the grand total of all chained DVE passes in acc[:, 0]
            fl = pool.tile([H, 1, 2], f32)
            dummy = dve_sc[:, 0:1, 0:2]  # RAW dep: runs after the whole chain
            _emit_abs(nc, cont_op, fl[:], dummy, dummy, acc[:, 0:1])
            nc.sync.dma_start(out[:], acc[:])

    nc.compile()
    return nc, "out"


def _make_slab(x: np.ndarray, y: np.ndarray, b: int, d0: int) -> np.ndarray:
    """[H, 2(half), 2(x|y), SLAB_S/2, SLAB_W] fp32 slab with halo + W pad,
    laid out so each DMA chunk is one contiguous run per partition."""
    slab = np.zeros((H, 2, 2, SLAB_S // 2, SLAB_W), dtype=np.float32)
    lo, hi = d0 - 1, d0 + D_CHUNK + 1
    clo, chi = max(lo, 0), min(hi, D)
    half = SLAB_S // 2
    for t, full in ((0, x), (1, y)):
        chunk = full[b, 0, clo:chi]  # [n, H, W]
        flat = np.zeros((H, SLAB_S, SLAB_W), dtype=np.float32)
        flat[:, clo - lo : chi - lo, 2 : 2 + W] = np.transpose(chunk, (1, 0, 2))
        slab[:, 0, t] = flat[:, :half]
        slab[:, 1, t] = flat[:, half:]
    return slab


def _make_in_maps(x: np.ndarray, y: np.ndarray) -> list:
    x = np.asarray(x, dtype=np.float32)
    y = np.asarray(y, dtype=np.float32)
    in_maps = []
    for core in range(N_CORES):
        b, chunk = divmod(core, 4)
        d0 = chunk * D_CHUNK
        in_maps.append({"xy": _make_slab(x, y, b, d0)})
    return in_maps


def _gamma_tables() -> np.ndarray:
    """[N_CORES, 9, H] float64 gamma weights for the device's 9 region sums.

    gamma(v) = w(v) - 2*u'(v): w = #offsets (of 26) whose partner exits the
    padded volume; u' = #canonical passes in which v contributed an |d(v)|
    term on device (partner exits in d or w; centers restricted to h<=126
    for oh=+1 passes)."""
    gam = np.zeros((N_CORES, 9, H))
    hs = np.arange(H)
    for core in range(N_CORES):
        chunk = core % 4
        d_reps = (chunk * D_CHUNK, chunk * D_CHUNK + 1, chunk * D_CHUNK + 7)
        w_reps = (0, 1, 127)
        for r in range(9):
            dd = d_reps[r // 3]
            ww = w_reps[r % 3]
            wcnt = np.zeros(H)
            ucnt = np.zeros(H)
            for od in (-1, 0, 1):
                for oh in (-1, 0, 1):
                    for ow in (-1, 0, 1):
                        if od == oh == ow == 0:
                            continue
                        exits = (
                            (not 0 <= dd + od < D)
                            | (hs + oh < 0)
                            | (hs + oh >= H)
                            | (not 0 <= ww + ow < W)
                        )
                        wcnt += exits
            for od, oh, ow in PASSES:
                dw_exit = (not 0 <= dd + od < D) or (not 0 <= ww + ow < W)
                if not dw_exit:
                    continue
                elig = np.ones(H, dtype=bool)
                if oh == 1:
                    elig = hs <= H - 2
                ucnt += elig
            gam[core, r] = wcnt - 2 * ucnt
    return gam


_GAMMA = None


def kernel(x: np.ndarray, y: np.ndarray) -> np.ndarray:
    global _cached, _GAMMA
    if _cached is None:
        _cached = _build()
        _GAMMA = _gamma_tables()
    nc, out_name = _cached

    from concourse.bass_utils import run_bass_kernel_spmd

    in_maps = _make_in_maps(x, y)
    res = run_bass_kernel_spmd(nc, in_maps, core_ids=list(range(N_CORES)))

    total = np.float64(0.0)
    for core in range(N_CORES):
        r = res.results[core][out_name].astype(np.float64)  # [H, 10]
        total += 2.0 * r[:, 0].sum()
        total += (_GAMMA[core] * r[:, 1:].T).sum()
    return np.asarray(total / TOTAL_COUNT, dtype=np.float32)


# revision 10
# speedup vs baseline: 1.6822x; 1.6822x over previous
"""Central-difference L1 loss kernel for 8 trn2 NeuronCores.

Math: with d = x - y, the loss is
    mean_{27 offsets o} |d[v] - d_pad[v + o]|
over the (B,C,D,H,W) = (2,1,32,128,128) volume, zero-padded by 1 in D/H/W.

Offset symmetry: |d[v] - d[v+o]| is counted by both o and -o, so only 13
canonical offsets (first nonzero of (oh, od, ow) positive) are computed on
device; total = 2 * sum(canonical directed sums) + sum_v gamma(v) * |d[v]|,
where gamma is a small integer boundary weight, constant on the 27 cells of
(d-class, h-class, w-class). The device also emits 9 per-partition |d|
region sums (slice-class x w-class; h = partition gives free h resolution);
the host applies gamma and folds in float64.

Sharding: 8 shards over (B=2) x (D in 4 chunks of 8 slices). Each core gets a
[128(H), 2(x|y), 10(slices incl halo), 132(W incl pad)] fp32 slab with zeros
in halo/pad positions that fall outside the volume.

Device per core (bf16 pipeline):
  d0   = x - y  (bf16; DVE tensor_tensor, one per slab half)
  d0s  = d0 shifted by one flat element (ScalarE copy; keeps the packed-bf16
         pair alignment so the custom DVE op runs its 2x_1p mode for w+-1)
  d0h  = d0 shifted by one partition (SBUF->SBUF DMA; h+1 neighbor)
  d0sh = d0s shifted by one partition (SBUF->SBUF DMA)
  13 canonical passes, one custom DVE op each (ABS_DIFF_ACC, hardware
  accumulator chained across ops), + 1 tiny flush op -> acc[:, 0]
  9 ScalarE activation(Abs) region sums with accum_out -> acc[:, 1..9]
Host folds 8 x [128, 10] partials with gamma weights in float64.
"""

import numpy as np

# ---- problem constants (hardcoded; kernel.py must be self-contained) ----
B, C, D, H, W = 2, 1, 32, 128, 128
N_CORES = 8
D_CHUNK = D // 4  # 8 slices per core
SLAB_S = D_CHUNK + 2  # with halo
SLAB_W = W + 4  # W + 2 pad each side (keeps slice stride & data start even)
FLAT = SLAB_S * SLAB_W
HALF = (SLAB_S // 2) * SLAB_W  # flat size of one slab half (5 slices)
N_OFFSETS = 27
TOTAL_COUNT = N_OFFSETS * B * C * D * H * W

# canonical offsets (od, oh, ow): first nonzero of (oh, od, ow) is +1,
# ordered by buffer readiness: d0 -> d0s -> d0h -> d0sh
PASSES = (
    [(1, 0, 0)]
    + [(1, 0, -1), (1, 0, 1), (0, 0, 1)]
    + [(-1, 1, 0), (0, 1, 0), (1, 1, 0)]
    + [(od, 1, ow) for od in (-1, 0, 1) for ow in (-1, 1)]
)
assert len(PASSES) == 13

_cached = None
_ABS_OP = None


def _register_abs_diff_op():
    """Register two custom DVE op rows:
      ABS2X_SEED: seed (acc <- 0) + steady; ABS2X_CONT: steady only (the
    hardware accumulator keeps integrating across instructions).
    Steady body (both rows, both modes) uses the native v3 ABSOLUTE_DIFF op:
      1x: |a - b| per element; 2x: |a-b| of the packed lo+hi bf16 pair summed.
    Machine shape throughout: accumulate recurrence early (CURR_ALU_OUT), acc
    rides the BYPASS chain with a_flop re-latched on every block to the end;
    DVE_READ_ACCUMULATOR2 taps that chain. The read only decodes correctly
    when the op's dst dtype is fp32, so the hot bf16 passes skip accum_out and
    a final tiny fp32-dst flush op (in0 == in1, adds 0) extracts the total."""
    global _ABS_OP
    if _ABS_OP is not None:
        return _ABS_OP
    from dataclasses import dataclass
    from operator import add

    import concourse.dve_ops as dve_ops
    from concourse.dve_ops import OPS, CUSTOM_DVE_SPECS, DveOp
    from concourse.dve_spec import Spec, Src0, Src1, lower, maxx
    from concourse.dve_uop import (
        AluInp,
        AluOp,
        DelayInp,
        DveOpSpec,
        InpSel,
        OutPath,
        OutSel,
        Trigger,
        UopConfig,
        UopDpConfig,
    )

    def _ref(in0, in1, s0, s1, imm2):
        b = np.abs(in0.astype(np.float32) - in1.astype(np.float32))
        return b, b.reshape(b.shape[0], -1).sum(axis=-1, keepdims=True)

    spec = Spec(body=maxx(Src0 - Src1, Src1 - Src0), accum=add, reference=_ref)

    PA, CA = AluInp.PREV_ALU_OUT, AluInp.CURR_ALU_OUT
    PD = lambda n: AluInp(int(AluInp.PREV_DELAY_0) + n)

    def mk_uop(kind, two_x):
        INP = [
            InpSel.SRC_0,
            InpSel.SRC_1,
            InpSel.SRC_0_HI if two_x else InpSel.ZERO,
            InpSel.SRC_1_HI if two_x else InpSel.ZERO,
        ] + [InpSel.ZERO] * 4
        INP_EN = ([1, 1, 1, 1] if two_x else [1, 1, 0, 0]) + [0, 0, 0, 0]
        bs = []
        for _ in range(8):
            b = UopDpConfig()
            b.op, b.alu_src0, b.alu_src1 = AluOp.BYPASS, PA, PA
            b.alu_out_enable = 1
            bs.append(b)

        def alu(i, op, s0, s1):
            bs[i].op, bs[i].alu_src0, bs[i].alu_src1 = op, s0, s1

        def chain(i, n, src=DelayInp.PREV_DELAY):
            bs[i].delay[n] = src
            bs[i].delay_enable[n] = 1

        if kind == "seed":
            acc_stage = 3
            alu(3, AluOp.BITWISE_XOR, PA, PA)  # acc <- 0
        elif two_x:
            acc_stage = 3
            alu(0, AluOp.ABSOLUTE_DIFF, PA, PD(0))  # |a_lo - b_lo|
            alu(1, AluOp.ABSOLUTE_DIFF, PD(1), PD(2))  # |a_hi - b_hi|
            alu(2, AluOp.ADD, PA, PD(3))  # pair sum
            alu(3, AluOp.ADD, CA, PA)  # accumulate
            chain(0, 1)  # a_hi to blk1
            chain(0, 2)  # b_hi to blk1
            chain(1, 3, DelayInp.PREV_ALU_OUT)  # chain3 <- |d_lo|
            chain(3, 0, DelayInp.PREV_ALU_OUT)  # chain0 <- body (for out)
            for i in (4, 5, 6, 7):
                chain(i, 0)
        else:
            # accum stage MUST match the 2x program (block 3): the running
            # total lives in that block's out-flop across chained ops, and a
            # mode-mismatched op in the chain must find it in the same place
            acc_stage = 3
            alu(0, AluOp.ABSOLUTE_DIFF, PA, PD(0))  # |a - b|
            alu(3, AluOp.ADD, CA, PA)  # accumulate
            chain(3, 0, DelayInp.PREV_ALU_OUT)  # chain0 <- body (for out)
            for i in (4, 5, 6, 7):
                chain(i, 0)
        for i in range(acc_stage, 8):
            bs[i].alu_out_a_enable = 1
        u = UopConfig(
            datapath_config=bs,
            inp=list(INP),
            inp_enable=list(INP_EN),
            accum_enabled=1,
            require_inp0=0 if kind == "seed" else 1,
            require_inp1=0 if kind == "seed" else 1,
            trigger=(
                (Trigger.COUNT, Trigger.NONE, Trigger.NONE)
                if kind == "seed"
                else (Trigger.SRC_TENSOR_DONE, Trigger.NONE, Trigger.NONE)
            ),
            next_uop=(1, 0, 0) if kind == "seed" else (0, 0, 0),
            repeat_count=1 if kind == "seed" else 0,
        )
        if kind != "seed":
            u.out[OutPath.WR0_LO] = OutSel.DELAY_0
            u.out_enable[OutPath.WR0_LO] = 1
            if two_x:
                u.out[OutPath.WR0_HI] = OutSel.DELAY_0
                u.out_enable[OutPath.WR0_HI] = 1
        return u

    def register(name, with_seed):
        row = max(dve_ops._SUB_OPCODE_FOR_NAME.values()) + 1
        assert row < 0x20
        dve_ops._SUB_OPCODE_FOR_NAME[name] = row

        if with_seed:
            u1 = [mk_uop("seed", False), mk_uop("steady", False)]
            u2 = [mk_uop("seed", True), mk_uop("steady", True)]
        else:
            u1 = [mk_uop("steady", False)]
            u2 = [mk_uop("steady", True)]

        @dataclass(frozen=True)
        class DveOpHand(DveOp):
            def compile(self, ver):
                key = (self.name, ver)
                if (r := dve_ops._COMPILE_CACHE.get(key)) is not None:
                    return r
                if ver == "v3":
                    r = DveOpSpec(
                        name=self.name, opcode=row, uops=u1, uops_2x=u2,
                        rd1_en=True, perf_max=1,
                    )
                else:
                    r = DveOpSpec(
                        name=self.name, opcode=row,
                        uops=lower(spec, ver=ver), rd1_en=True,
                    )
                dve_ops._COMPILE_CACHE[key] = r
                return r

        op = DveOpHand(name, spec, subdim=False, uops_sha={})
        OPS.append(op)
        CUSTOM_DVE_SPECS[name] = spec
        return op

    _ABS_OP = (register("ABS2X_SEED_V7_ANT", True), register("ABS2X_CONT_V7_ANT", False))
    return _ABS_OP


def _emit_abs(nc, op, out, in0, in1, accum_out=None, s0=0.0):
    """_custom_dve clone that sets perf_max=1 (byte-36[7:6]) so the engine
    picks the 2x_1p uop slot when the APs qualify (silent 1x fallback)."""
    import concourse.bass_isa as bass_isa
    from concourse import mybir
    from concourse.dve_ops import get_dve_sub_opcode

    v = nc.vector
    if op.name not in nc.m.ant_custom_dve_ops:
        nc.m.ant_custom_dve_ops = sorted({*nc.m.ant_custom_dve_ops, op.name})
    shape = bass_isa.CustomDveShape.STT
    isa_opcode = nc.isa.Opcode[
        f"NEURON_ISA_TPB_OPCODE_CUSTOM_DVE_ANT_{shape.slot()}"
    ].value
    zero = mybir.ImmediateValue(dtype=mybir.dt.float32, value=0.0)
    s0_l = v.lower_ap(s0, for_isa=True) if not isinstance(s0, float) else zero
    ins = [
        v.lower_ap(in0, for_isa=True, opt=True),
        v.lower_ap(in1, for_isa=True, opt=True),
        s0_l,
        zero,
    ]
    outs = [v.lower_ap(out, for_isa=True, opt=True)]
    if accum_out is not None:
        outs.append(v.lower_ap(accum_out, for_isa=True))
    return v.add_instruction(
        bass_isa.InstCustomDveAnt(
            name=nc.get_next_instruction_name(),
            op_name=op.name,
            rd1_en=True,
            subdim=0,
            imm2=0.0,
            shape=shape,
            row=get_dve_sub_opcode(op.name),
            isa_opcode=isa_opcode,
            ins=ins,
            outs=outs,
            perf_max=1,
        )
    )


def _build():
    """Build and schedule the Bass program once; return (nc, out_name)."""
    import concourse.tile as tile
    from concourse import bacc, mybir

    seed_op, cont_op = _register_abs_diff_op()
    f32 = mybir.dt.float32
    bf16 = mybir.dt.bfloat16
    AF = mybir.ActivationFunctionType
    nc = bacc.Bacc(
        "TRN2",
        target_bir_lowering=False,
        debug=False,
        enable_asserts=False,
        num_devices=N_CORES,
    )
    # chunk-major DRAM layout: each [k, t] chunk is one linear 338KB run, so
    # the DMA lowers to a clean pattern instead of per-partition descriptors
    xy = nc.dram_tensor("xy", [2, 2, H, SLAB_S // 2, SLAB_W], f32, kind="ExternalInput").ap()
    shm = nc.dram_tensor("shm", [H, H], bf16, kind="ExternalInput").ap()
    out = nc.dram_tensor("out", [H, 10], f32, kind="ExternalOutput").ap()

    with tile.TileContext(nc) as tc:
        with (
            tc.tile_pool(name="main", bufs=1) as pool,
            tc.tile_pool(name="psum", bufs=2, space="PSUM") as psum_pool,
        ):
            xyt = pool.tile([H, 2, 2, SLAB_S // 2, SLAB_W], f32)
            sh = pool.tile([H, H], bf16)
            d0 = pool.tile([H, SLAB_S, SLAB_W], bf16)
            d0s = pool.tile([H, SLAB_S, SLAB_W], bf16)
            d0h = pool.tile([H, SLAB_S, SLAB_W], bf16)
            d0sh = pool.tile([H, SLAB_S, SLAB_W], bf16)
            acc = pool.tile([H, 10], f32)
            dve_sc = pool.tile([H, D_CHUNK, W], bf16)  # shared scrap: WAW chain
            reg_sc = pool.tile([H, SLAB_S - 4, W - 2], bf16)  # scalar scrap

            # input DMAs: 4 chunks spread across engine DGE rings so the
            # transfers run in parallel
            nc.sync.dma_start(xyt[:, 0, 0], xy[0, 0])
            nc.scalar.dma_start(xyt[:, 0, 1], xy[0, 1])
            nc.gpsimd.dma_start(xyt[:, 1, 0], xy[1, 0])
            nc.sync.dma_start(xyt[:, 1, 1], xy[1, 1])
            nc.gpsimd.dma_start(sh[:], shm[:])

            half = SLAB_S // 2
            for k, (s0_, s1_) in enumerate(((0, half), (half, SLAB_S))):
                nc.vector.tensor_tensor(
                    out=d0[:, s0_:s1_],
                    in0=xyt[:, k, 0],
                    in1=xyt[:, k, 1],
                    op=mybir.AluOpType.subtract,
                )
            d0f = d0[:].rearrange("p a b -> p (a b)")
            d0sf = d0s[:].rearrange("p a b -> p (a b)")

            # d0s: flat shift by one element (ScalarE), split per slab half so
            # the first chunk only waits on d0's first half
            nc.scalar.copy(d0sf[:, 0 : HALF - 1], d0f[:, 1:HALF])
            nc.scalar.copy(d0sf[:, HALF - 1 : FLAT - 1], d0f[:, HALF:FLAT])

            # h+1 twins (twin[p] = src[p+1], zero row 127) via TensorE with the
            # sub-diagonal shift matrix; PSUM -> bf16 SBUF casts on ScalarE
            twins = {}
            for key, srcf in (("d0h", d0f), ("d0sh", d0sf)):
                ps = psum_pool.tile([H, FLAT], f32, tag="ps")
                for c0 in range(0, FLAT, 512):
                    c1 = min(c0 + 512, FLAT)
                    nc.tensor.matmul(
                        ps[:, c0:c1], sh[:], srcf[:, c0:c1], start=True, stop=True
                    )
                t = pool.tile([H, SLAB_S, SLAB_W], bf16, name=key)
                tf = t[:].rearrange("p a b -> p (a b)")
                nc.scalar.copy(tf[:], ps[:])
                twins[key] = t
            d0h, d0sh = twins["d0h"], twins["d0sh"]

            # 13 canonical passes on the DVE, one hardware-accumulator chain
            for i, (od, oh, ow) in enumerate(PASSES):
                if oh == 1:
                    twin = d0h if ow == 0 else d0sh
                else:
                    twin = d0 if ow == 0 else d0s
                c0, c1 = (2, 130) if ow >= 0 else (0, 128)
                in0 = d0[:, 1:9, 2:130]
                in1 = twin[:, 1 + od : 9 + od, c0:c1]
                op = seed_op if i == 0 else cont_op
                _emit_abs(nc, op, dve_sc[:], in0, in1)

            # 9 |d| region sums on ScalarE (slice-class x w-class), per-
            # partition accumulators; host applies gamma weights
            ridx = 0
            for s0_, s1_ in ((1, 2), (2, 8), (8, 9)):
                for c0, c1 in ((2, 3), (3, 129), (129, 130)):
                    nc.scalar.activation(
                        reg_sc[:, 0 : s1_ - s0_, 0 : c1 - c0],
                        d0[:, s0_:s1_, c0:c1],
                        AF.Abs,
                        accum_out=acc[:, 1 + ridx : 2 + ridx],
                    )
                    ridx += 1

            # flush: tiny fp32-dst continue op; in0 == in1 adds 0; its
            # appended accumulator read decodes correctly (fp32) and lands
            # the grand total of all chained DVE passes in acc[:, 0]
            fl = pool.tile([H, 1, 2], f32)
            dummy = dve_sc[:, 0:1, 0:2]  # RAW dep: runs after the whole chain
            _emit_abs(nc, cont_op, fl[:], dummy, dummy, acc[:, 0:1])
            nc.sync.dma_start(out[:], acc[:])

    nc.compile()
    return nc, "out"


def _make_slab(x: np.ndarray, y: np.ndarray, b: int, d0: int) -> np.ndarray:
    """[2(half), 2(x|y), H, SLAB_S/2, SLAB_W] fp32 slab with halo + W pad;
    chunk-major so each DMA chunk is one linear DRAM run."""
    slab = np.zeros((2, 2, H, SLAB_S // 2, SLAB_W), dtype=np.float32)
    lo, hi = d0 - 1, d0 + D_CHUNK + 1
    clo, chi = max(lo, 0), min(hi, D)
    half = SLAB_S // 2
    for t, full in ((0, x), (1, y)):
        chunk = full[b, 0, clo:chi]  # [n, H, W]
        flat = np.zeros((H, SLAB_S, SLAB_W), dtype=np.float32)
        flat[:, clo - lo : chi - lo, 2 : 2 + W] = np.transpose(chunk, (1, 0, 2))
        slab[0, t] = flat[:, :half]
        slab[1, t] = flat[:, half:]
    return slab


def _make_in_maps(x: np.ndarray, y: np.ndarray) -> list:
    import ml_dtypes

    x = np.asarray(x, dtype=np.float32)
    y = np.asarray(y, dtype=np.float32)
    shm = np.eye(H, k=-1, dtype=np.float32).astype(ml_dtypes.bfloat16)
    in_maps = []
    for core in range(N_CORES):
        b, chunk = divmod(core, 4)
        d0 = chunk * D_CHUNK
        in_maps.append({"xy": _make_slab(x, y, b, d0), "shm": shm})
    return in_maps


def _gamma_tables() -> np.ndarray:
    """[N_CORES, 9, H] float64 gamma weights for the device's 9 region sums.

    gamma(v) = w(v) - 2*u'(v): w = #offsets (of 26) whose partner exits the
    padded volume; u' = #canonical passes in which v contributed an |d(v)|
    term on device (partner exits in d, h, or w — the shift-matmul twins have
    a zero row 127, so the passes follow full zero-pad semantics)."""
    gam = np.zeros((N_CORES, 9, H))
    hs = np.arange(H)
    for core in range(N_CORES):
        chunk = core % 4
        d_reps = (chunk * D_CHUNK, chunk * D_CHUNK + 1, chunk * D_CHUNK + 7)
        w_reps = (0, 1, 127)
        for r in range(9):
            dd = d_reps[r // 3]
            ww = w_reps[r % 3]
            wcnt = np.zeros(H)
            ucnt = np.zeros(H)
            for od in (-1, 0, 1):
                for oh in (-1, 0, 1):
                    for ow in (-1, 0, 1):
                        if od == oh == ow == 0:
                            continue
                        exits = (
                            (not 0 <= dd + od < D)
                            | (hs + oh < 0)
                            | (hs + oh >= H)
                            | (not 0 <= ww + ow < W)
                        )
                        wcnt += exits
            for od, oh, ow in PASSES:
                exit_v = (
                    (not 0 <= dd + od < D)
                    | (hs + oh < 0)
                    | (hs + oh >= H)
                    | (not 0 <= ww + ow < W)
                )
                ucnt += exit_v if isinstance(exit_v, np.ndarray) else (
                    np.full(H, exit_v, dtype=float)
                )
            gam[core, r] = wcnt - 2 * ucnt
    return gam


_GAMMA = None


def kernel(x: np.ndarray, y: np.ndarray) -> np.ndarray:
    global _cached, _GAMMA
    if _cached is None:
        _cached = _build()
        _GAMMA = _gamma_tables()
    nc, out_name = _cached

    from concourse.bass_utils import run_bass_kernel_spmd

    in_maps = _make_in_maps(x, y)
    res = run_bass_kernel_spmd(nc, in_maps, core_ids=list(range(N_CORES)))

    total = np.float64(0.0)
    for core in range(N_CORES):
        r = res.results[core][out_name].astype(np.float64)  # [H, 10]
        total += 2.0 * r[:, 0].sum()
        total += (_GAMMA[core] * r[:, 1:].T).sum()
    return np.asarray(total / TOTAL_COUNT, dtype=np.float32)


# revision 16
# speedup vs baseline: 1.7413x; 1.0351x over previous
"""Central-difference L1 loss kernel for 8 trn2 NeuronCores.

Math: with d = x - y, the loss is
    mean_{27 offsets o} |d[v] - d_pad[v + o]|
over the (B,C,D,H,W) = (2,1,32,128,128) volume, zero-padded by 1 in D/H/W.

Offset symmetry: |d[v] - d[v+o]| is counted by both o and -o, so only 13
canonical offsets (first nonzero of (oh, od, ow) positive) are computed on
device; total = 2 * sum(canonical directed sums) + sum_v gamma(v) * |d[v]|,
where gamma is a small integer boundary weight, constant on the 27 cells of
(d-class, h-class, w-class). The device also emits 9 per-partition |d|
region sums (slice-class x w-class; h = partition gives free h resolution);
the host applies gamma and folds in float64.

Sharding: 8 shards over (B=2) x (D in 4 chunks of 8 slices). Each core gets a
[128(H), 2(x|y), 10(slices incl halo), 132(W incl pad)] fp32 slab with zeros
in halo/pad positions that fall outside the volume.

Device per core (bf16 pipeline):
  d0   = x - y  (bf16; DVE tensor_tensor, one per slab half)
  d0s  = d0 shifted by one flat element (ScalarE copy; keeps the packed-bf16
         pair alignment so the custom DVE op runs its 2x_1p mode for w+-1)
  d0h  = d0 shifted by one partition (SBUF->SBUF DMA; h+1 neighbor)
  d0sh = d0s shifted by one partition (SBUF->SBUF DMA)
  13 canonical passes, one custom DVE op each (ABS_DIFF_ACC, hardware
  accumulator chained across ops), + 1 tiny flush op -> acc[:, 0]
  9 ScalarE activation(Abs) region sums with accum_out -> acc[:, 1..9]
Host folds 8 x [128, 10] partials with gamma weights in float64.
"""

import numpy as np

# ---- problem constants (hardcoded; kernel.py must be self-contained) ----
B, C, D, H, W = 2, 1, 32, 128, 128
N_CORES = 8
D_CHUNK = D // 4  # 8 slices per core
SLAB_S = D_CHUNK + 2  # with halo
SLAB_W = W + 4  # W + 2 pad each side (keeps slice stride & data start even)
FLAT = SLAB_S * SLAB_W
HALF = (SLAB_S // 2) * SLAB_W  # flat size of one slab half (5 slices)
N_OFFSETS = 27
TOTAL_COUNT = N_OFFSETS * B * C * D * H * W

# canonical offsets (od, oh, ow): first nonzero of (oh, od, ow) is +1,
# ordered by buffer readiness: d0 -> d0s -> d0h -> d0sh
PASSES = (
    [(1, 0, 0)]
    + [(1, 0, -1), (1, 0, 1), (0, 0, 1)]
    + [(-1, 1, 0), (0, 1, 0), (1, 1, 0)]
    + [(od, 1, ow) for od in (-1, 0, 1) for ow in (-1, 1)]
)
assert len(PASSES) == 13

_cached = None
_ABS_OP = None


def _register_abs_diff_op():
    """Register two custom DVE op rows:
      ABS2X_SEED: seed (acc <- 0) + steady; ABS2X_CONT: steady only (the
    hardware accumulator keeps integrating across instructions).
    Steady body (both rows, both modes) uses the native v3 ABSOLUTE_DIFF op:
      1x: |a - b| per element; 2x: |a-b| of the packed lo+hi bf16 pair summed.
    Machine shape throughout: accumulate recurrence early (CURR_ALU_OUT), acc
    rides the BYPASS chain with a_flop re-latched on every block to the end;
    DVE_READ_ACCUMULATOR2 taps that chain. The read only decodes correctly
    when the op's dst dtype is fp32, so the hot bf16 passes skip accum_out and
    a final tiny fp32-dst flush op (in0 == in1, adds 0) extracts the total."""
    global _ABS_OP
    if _ABS_OP is not None:
        return _ABS_OP
    from dataclasses import dataclass
    from operator import add

    import concourse.dve_ops as dve_ops
    from concourse.dve_ops import OPS, CUSTOM_DVE_SPECS, DveOp
    from concourse.dve_spec import Spec, Src0, Src1, lower, maxx
    from concourse.dve_uop import (
        AluInp,
        AluOp,
        DelayInp,
        DveOpSpec,
        InpSel,
        OutPath,
        OutSel,
        Trigger,
        UopConfig,
        UopDpConfig,
    )

    def _ref(in0, in1, s0, s1, imm2):
        b = np.abs(in0.astype(np.float32) - in1.astype(np.float32))
        return b, b.reshape(b.shape[0], -1).sum(axis=-1, keepdims=True)

    spec = Spec(body=maxx(Src0 - Src1, Src1 - Src0), accum=add, reference=_ref)

    PA, CA = AluInp.PREV_ALU_OUT, AluInp.CURR_ALU_OUT
    PD = lambda n: AluInp(int(AluInp.PREV_DELAY_0) + n)

    def mk_uop(kind, two_x):
        INP = [
            InpSel.SRC_0,
            InpSel.SRC_1,
            InpSel.SRC_0_HI if two_x else InpSel.ZERO,
            InpSel.SRC_1_HI if two_x else InpSel.ZERO,
        ] + [InpSel.ZERO] * 4
        INP_EN = ([1, 1, 1, 1] if two_x else [1, 1, 0, 0]) + [0, 0, 0, 0]
        bs = []
        for _ in range(8):
            b = UopDpConfig()
            b.op, b.alu_src0, b.alu_src1 = AluOp.BYPASS, PA, PA
            b.alu_out_enable = 1
            bs.append(b)

        def alu(i, op, s0, s1):
            bs[i].op, bs[i].alu_src0, bs[i].alu_src1 = op, s0, s1

        def chain(i, n, src=DelayInp.PREV_DELAY):
            bs[i].delay[n] = src
            bs[i].delay_enable[n] = 1

        if kind == "seed":
            acc_stage = 3
            alu(3, AluOp.BITWISE_XOR, PA, PA)  # acc <- 0
        elif two_x:
            acc_stage = 3
            alu(0, AluOp.ABSOLUTE_DIFF, PA, PD(0))  # |a_lo - b_lo|
            alu(1, AluOp.ABSOLUTE_DIFF, PD(1), PD(2))  # |a_hi - b_hi|
            alu(2, AluOp.ADD, PA, PD(3))  # pair sum
            alu(3, AluOp.ADD, CA, PA)  # accumulate
            chain(0, 1)  # a_hi to blk1
            chain(0, 2)  # b_hi to blk1
            chain(1, 3, DelayInp.PREV_ALU_OUT)  # chain3 <- |d_lo|
            chain(3, 0, DelayInp.PREV_ALU_OUT)  # chain0 <- body (for out)
            for i in (4, 5, 6, 7):
                chain(i, 0)
        else:
            # accum stage MUST match the 2x program (block 3): the running
            # total lives in that block's out-flop across chained ops, and a
            # mode-mismatched op in the chain must find it in the same place
            acc_stage = 3
            alu(0, AluOp.ABSOLUTE_DIFF, PA, PD(0))  # |a - b|
            alu(3, AluOp.ADD, CA, PA)  # accumulate
            chain(3, 0, DelayInp.PREV_ALU_OUT)  # chain0 <- body (for out)
            for i in (4, 5, 6, 7):
                chain(i, 0)
        for i in range(acc_stage, 8):
            bs[i].alu_out_a_enable = 1
        u = UopConfig(
            datapath_config=bs,
            inp=list(INP),
            inp_enable=list(INP_EN),
            accum_enabled=1,
            require_inp0=0 if kind == "seed" else 1,
            require_inp1=0 if kind == "seed" else 1,
            trigger=(
                (Trigger.COUNT, Trigger.NONE, Trigger.NONE)
                if kind == "seed"
                else (Trigger.SRC_TENSOR_DONE, Trigger.NONE, Trigger.NONE)
            ),
            next_uop=(1, 0, 0) if kind == "seed" else (0, 0, 0),
            repeat_count=1 if kind == "seed" else 0,
        )
        if kind != "seed":
            u.out[OutPath.WR0_LO] = OutSel.DELAY_0
            u.out_enable[OutPath.WR0_LO] = 1
            if two_x:
                u.out[OutPath.WR0_HI] = OutSel.DELAY_0
                u.out_enable[OutPath.WR0_HI] = 1
        return u

    def register(name, with_seed):
        row = max(dve_ops._SUB_OPCODE_FOR_NAME.values()) + 1
        assert row < 0x20
        dve_ops._SUB_OPCODE_FOR_NAME[name] = row

        if with_seed:
            u1 = [mk_uop("seed", False), mk_uop("steady", False)]
            u2 = [mk_uop("seed", True), mk_uop("steady", True)]
        else:
            u1 = [mk_uop("steady", False)]
            u2 = [mk_uop("steady", True)]

        @dataclass(frozen=True)
        class DveOpHand(DveOp):
            def compile(self, ver):
                key = (self.name, ver)
                if (r := dve_ops._COMPILE_CACHE.get(key)) is not None:
                    return r
                if ver == "v3":
                    r = DveOpSpec(
                        name=self.name, opcode=row, uops=u1, uops_2x=u2,
                        rd1_en=True, perf_max=1,
                    )
                else:
                    r = DveOpSpec(
                        name=self.name, opcode=row,
                        uops=lower(spec, ver=ver), rd1_en=True,
                    )
                dve_ops._COMPILE_CACHE[key] = r
                return r

        op = DveOpHand(name, spec, subdim=False, uops_sha={})
        OPS.append(op)
        CUSTOM_DVE_SPECS[name] = spec
        return op

    _ABS_OP = (register("ABS2X_SEED_V7_ANT", True), register("ABS2X_CONT_V7_ANT", False))
    return _ABS_OP


def _emit_abs(nc, op, out, in0, in1, accum_out=None, s0=0.0):
    """_custom_dve clone that sets perf_max=1 (byte-36[7:6]) so the engine
    picks the 2x_1p uop slot when the APs qualify (silent 1x fallback)."""
    import concourse.bass_isa as bass_isa
    from concourse import mybir
    from concourse.dve_ops import get_dve_sub_opcode

    v = nc.vector
    if op.name not in nc.m.ant_custom_dve_ops:
        nc.m.ant_custom_dve_ops = sorted({*nc.m.ant_custom_dve_ops, op.name})
    shape = bass_isa.CustomDveShape.STT
    isa_opcode = nc.isa.Opcode[
        f"NEURON_ISA_TPB_OPCODE_CUSTOM_DVE_ANT_{shape.slot()}"
    ].value
    zero = mybir.ImmediateValue(dtype=mybir.dt.float32, value=0.0)
    s0_l = v.lower_ap(s0, for_isa=True) if not isinstance(s0, float) else zero
    ins = [
        v.lower_ap(in0, for_isa=True, opt=True),
        v.lower_ap(in1, for_isa=True, opt=True),
        s0_l,
        zero,
    ]
    outs = [v.lower_ap(out, for_isa=True, opt=True)]
    if accum_out is not None:
        outs.append(v.lower_ap(accum_out, for_isa=True))
    return v.add_instruction(
        bass_isa.InstCustomDveAnt(
            name=nc.get_next_instruction_name(),
            op_name=op.name,
            rd1_en=True,
            subdim=0,
            imm2=0.0,
            shape=shape,
            row=get_dve_sub_opcode(op.name),
            isa_opcode=isa_opcode,
            ins=ins,
            outs=outs,
            perf_max=1,
        )
    )


def _build():
    """Build and schedule the Bass program once; return (nc, out_name)."""
    import concourse.tile as tile
    from concourse import bacc, mybir

    seed_op, cont_op = _register_abs_diff_op()
    f32 = mybir.dt.float32
    bf16 = mybir.dt.bfloat16
    AF = mybir.ActivationFunctionType
    nc = bacc.Bacc(
        "TRN2",
        target_bir_lowering=False,
        debug=False,
        enable_asserts=False,
        num_devices=N_CORES,
    )
    # chunk-major DRAM layout: each [k, t] chunk is one linear run, so the
    # DMA lowers to a clean pattern; bf16 inputs halve the transfer bytes
    xy = nc.dram_tensor("xy", [2, 2, H, SLAB_S // 2, SLAB_W], bf16, kind="ExternalInput").ap()
    shm = nc.dram_tensor("shm", [H, H], bf16, kind="ExternalInput").ap()
    out = nc.dram_tensor("out", [3, 10], f32, kind="ExternalOutput").ap()

    with tile.TileContext(nc) as tc:
        with (
            tc.tile_pool(name="main", bufs=1) as pool,
            tc.tile_pool(name="psum", bufs=2, space="PSUM") as psum_pool,
        ):
            xyt = pool.tile([H, 2, 2, SLAB_S // 2, SLAB_W], bf16)
            sh = pool.tile([H, H], bf16)
            d0 = pool.tile([H, SLAB_S, SLAB_W], bf16)
            d0s = pool.tile([H, SLAB_S, SLAB_W], bf16)
            d0h = pool.tile([H, SLAB_S, SLAB_W], bf16)
            d0sh = pool.tile([H, SLAB_S, SLAB_W], bf16)
            acc = pool.tile([H, 10], f32)
            dve_sc = pool.tile([H, D_CHUNK, W], bf16)  # shared scrap: WAW chain
            reg_sc = pool.tile([H, SLAB_S - 4, W - 2], bf16)  # scalar scrap

            # input DMAs: 4 chunks spread across engine DGE rings so the
            # transfers run in parallel
            nc.sync.dma_start(xyt[:, 0, 0], xy[0, 0])
            nc.scalar.dma_start(xyt[:, 0, 1], xy[0, 1])
            nc.gpsimd.dma_start(xyt[:, 1, 0], xy[1, 0])
            nc.sync.dma_start(xyt[:, 1, 1], xy[1, 1])
            nc.gpsimd.dma_start(sh[:], shm[:])

            half = SLAB_S // 2
            for k, (s0_, s1_) in enumerate(((0, half), (half, SLAB_S))):
                nc.vector.tensor_tensor(
                    out=d0[:, s0_:s1_],
                    in0=xyt[:, k, 0],
                    in1=xyt[:, k, 1],
                    op=mybir.AluOpType.subtract,
                )
            d0f = d0[:].rearrange("p a b -> p (a b)")
            d0sf = d0s[:].rearrange("p a b -> p (a b)")

            # d0s: flat shift by one element (ScalarE), split per slab half so
            # the first chunk only waits on d0's first half
            nc.scalar.copy(d0sf[:, 0 : HALF - 1], d0f[:, 1:HALF])
            nc.scalar.copy(d0sf[:, HALF - 1 : FLAT - 1], d0f[:, HALF:FLAT])

            # h+1 twins (twin[p] = src[p+1], zero row 127) via TensorE with the
            # sub-diagonal shift matrix; PSUM -> bf16 SBUF casts on ScalarE
            twins = {}
            for key, srcf in (("d0h", d0f), ("d0sh", d0sf)):
                ps = psum_pool.tile([H, FLAT], f32, tag="ps")
                for c0 in range(0, FLAT, 512):
                    c1 = min(c0 + 512, FLAT)
                    nc.tensor.matmul(
                        ps[:, c0:c1], sh[:], srcf[:, c0:c1], start=True, stop=True
                    )
                t = pool.tile([H, SLAB_S, SLAB_W], bf16, name=key)
                tf = t[:].rearrange("p a b -> p (a b)")
                nc.scalar.copy(tf[:], ps[:])
                twins[key] = t
            d0h, d0sh = twins["d0h"], twins["d0sh"]

            # 13 canonical passes on the DVE, one hardware-accumulator chain
            for i, (od, oh, ow) in enumerate(PASSES):
                if oh == 1:
                    twin = d0h if ow == 0 else d0sh
                else:
                    twin = d0 if ow == 0 else d0s
                c0, c1 = (2, 130) if ow >= 0 else (0, 128)
                in0 = d0[:, 1:9, 2:130]
                in1 = twin[:, 1 + od : 9 + od, c0:c1]
                op = seed_op if i == 0 else cont_op
                _emit_abs(nc, op, dve_sc[:], in0, in1)

            # 9 |d| region sums on ScalarE (slice-class x w-class), per-
            # partition accumulators; host applies gamma weights
            ridx = 0
            for s0_, s1_ in ((1, 2), (2, 8), (8, 9)):
                for c0, c1 in ((2, 3), (3, 129), (129, 130)):
                    nc.scalar.activation(
                        reg_sc[:, 0 : s1_ - s0_, 0 : c1 - c0],
                        d0[:, s0_:s1_, c0:c1],
                        AF.Abs,
                        accum_out=acc[:, 1 + ridx : 2 + ridx],
                    )
                    ridx += 1

            # flush: tiny fp32-dst continue op; in0 == in1 adds 0; its
            # appended accumulator read decodes correctly (fp32) and lands
            # the grand total of all chained DVE passes in acc[:, 0]
            fl = pool.tile([H, 1, 2], f32)
            dummy = dve_sc[:, 0:1, 0:2]  # RAW dep: runs after the whole chain
            _emit_abs(nc, cont_op, fl[:], dummy, dummy, acc[:, 0:1])

            # reduce acc over partitions on the PE (ones^T @ acc) so the out
            # DMA is 3 single-descriptor rows instead of 128; gamma only needs
            # h-resolution at rows 0 and 127 (interior h-classes are constant)
            ones = nc.const_aps.aps[(f32, 1.0)]
            ps_red = psum_pool.tile([H, 10], f32, tag="red")
            nc.tensor.matmul(ps_red[0:1, :], ones, acc[:], start=True, stop=True)
            red_sb = pool.tile([H, 10], f32)
            nc.scalar.copy(red_sb[0:1, :], ps_red[0:1, :])
            nc.sync.dma_start(out[0:1], red_sb[0:1])
            nc.sync.dma_start(out[1:2], acc[0:1])
            nc.scalar.dma_start(out[2:3], acc[127:128])

    nc.compile()
    return nc, "out"


def _make_slab(x: np.ndarray, y: np.ndarray, b: int, d0: int) -> np.ndarray:
    """[2(half), 2(x|y), H, SLAB_S/2, SLAB_W] fp32 slab with halo + W pad;
    chunk-major so each DMA chunk is one linear DRAM run."""
    import ml_dtypes

    slab = np.zeros((2, 2, H, SLAB_S // 2, SLAB_W), dtype=ml_dtypes.bfloat16)
    lo, hi = d0 - 1, d0 + D_CHUNK + 1
    clo, chi = max(lo, 0), min(hi, D)
    half = SLAB_S // 2
    for t, full in ((0, x), (1, y)):
        chunk = full[b, 0, clo:chi]  # [n, H, W]
        flat = np.zeros((H, SLAB_S, SLAB_W), dtype=ml_dtypes.bfloat16)
        flat[:, clo - lo : chi - lo, 2 : 2 + W] = np.transpose(chunk, (1, 0, 2))
        slab[0, t] = flat[:, :half]
        slab[1, t] = flat[:, half:]
    return slab


def _make_in_maps(x: np.ndarray, y: np.ndarray) -> list:
    import ml_dtypes

    x = np.asarray(x, dtype=np.float32)
    y = np.asarray(y, dtype=np.float32)
    shm = np.eye(H, k=-1, dtype=np.float32).astype(ml_dtypes.bfloat16)
    in_maps = []
    for core in range(N_CORES):
        b, chunk = divmod(core, 4)
        d0 = chunk * D_CHUNK
        in_maps.append({"xy": _make_slab(x, y, b, d0), "shm": shm})
    return in_maps


def _gamma_tables() -> np.ndarray:
    """[N_CORES, 9, H] float64 gamma weights for the device's 9 region sums.

    gamma(v) = w(v) - 2*u'(v): w = #offsets (of 26) whose partner exits the
    padded volume; u' = #canonical passes in which v contributed an |d(v)|
    term on device (partner exits in d, h, or w — the shift-matmul twins have
    a zero row 127, so the passes follow full zero-pad semantics)."""
    gam = np.zeros((N_CORES, 9, H))
    hs = np.arange(H)
    for core in range(N_CORES):
        chunk = core % 4
        d_reps = (chunk * D_CHUNK, chunk * D_CHUNK + 1, chunk * D_CHUNK + 7)
        w_reps = (0, 1, 127)
        for r in range(9):
            dd = d_reps[r // 3]
            ww = w_reps[r % 3]
            wcnt = np.zeros(H)
            ucnt = np.zeros(H)
            for od in (-1, 0, 1):
                for oh in (-1, 0, 1):
                    for ow in (-1, 0, 1):
                        if od == oh == ow == 0:
                            continue
                        exits = (
                            (not 0 <= dd + od < D)
                            | (hs + oh < 0)
                            | (hs + oh >= H)
                            | (not 0 <= ww + ow < W)
                        )
                        wcnt += exits
            for od, oh, ow in PASSES:
                exit_v = (
                    (not 0 <= dd + od < D)
                    | (hs + oh < 0)
                    | (hs + oh >= H)
                    | (not 0 <= ww + ow < W)
                )
                ucnt += exit_v if isinstance(exit_v, np.ndarray) else (
                    np.full(H, exit_v, dtype=float)
                )
            gam[core, r] = wcnt - 2 * ucnt
    return gam


_GAMMA = None


def kernel(x: np.ndarray, y: np.ndarray) -> np.ndarray:
    global _cached, _GAMMA
    if _cached is None:
        _cached = _build()
        _GAMMA = _gamma_tables()
    nc, out_name = _cached

    from concourse.bass_utils import run_bass_kernel_spmd

    in_maps = _make_in_maps(x, y)
    res = run_bass_kernel_spmd(nc, in_maps, core_ids=list(range(N_CORES)))

    total = np.float64(0.0)
    for core in range(N_CORES):
        r = res.results[core][out_name].astype(np.float64)  # [3, 10]
        colsum, row0, row127 = r[0], r[1], r[2]
        total += 2.0 * colsum[0]
        g = _GAMMA[core]  # [9, H]
        for reg in range(9):
            g_int, g0, g127 = g[reg, 1], g[reg, 0], g[reg, 127]
            s_all, s0, s127 = colsum[1 + reg], row0[1 + reg], row127[1 + reg]
            total += g_int * (s_all - s0 - s127) + g0 * s0 + g127 * s127
    return np.asarray(total / TOTAL_COUNT, dtype=np.float32)
